# revision 1
# baseline (speedup 1.0000x reference)
"""LoRA gather-BMM + dense GEMM kernel for Trainium2 (8 NeuronCores).

Computation (per the module semantics):
    A = lora_A[wids]; Bw = lora_B[wids]
    y = (x @ A) @ Bw * 2 + x @ M          # x: [B, 1, IN]

Distribution: data-parallel over batch. Each of the 8 cores processes
B/8 = 256 samples and reads the full (small) adapter banks and M.
No collectives; per-core outputs are concatenated on the host.

Per-core algorithm (all PE work in fp16 with fp32 PSUM accumulation):
  1. H^T = A_all^T @ x^T   for ALL 64 adapters  -> [1024, 256] (rank-major)
  2. h^T = H^T * mask      where mask[p, j] = (wids[j] == row_adapter[p]);
     this realizes the gather densely (row_adapter is an iota constant).
  3. y   = x @ M + h_exp @ (2 * B_all)   accumulated in PSUM, drained to fp16.
"""

import numpy as np

import concourse.bacc as bacc
import concourse.mybir as mybir
import concourse.tile as tile
from concourse.bass_utils import run_bass_kernel_spmd

B, IN, R, OUT, NA = 2048, 4096, 16, 4096, 64
N_CORES = 8
BC = B // N_CORES          # 256 samples per core
P = 128
KT = IN // P               # 32 contraction tiles over IN
NR = NA * R                # 1024 stacked rank rows
RT = NR // P               # 8 contraction tiles over rank
NH = 2                     # halves of OUT per PSUM pass
HW = OUT // NH             # 2048
NS = HW // 512             # 4 free-dim slices of 512
MB = BC // P               # 2 batch tiles

F16 = mybir.dt.float16
F32 = mybir.dt.float32


def build_nc(loop_n=None, staggered=False):
    nc = bacc.Bacc(
        "TRN2",
        target_bir_lowering=False,
        debug=False,
        enable_asserts=False,
        num_devices=N_CORES,
    )

    xt = nc.dram_tensor("xt", [P, KT, BC], F16, kind="ExternalInput")
    wd = nc.dram_tensor("wd", [P, BC], F16, kind="ExternalInput")
    ra = nc.dram_tensor("ra", [P, RT], F32, kind="ExternalInput")
    aal = nc.dram_tensor("aal", [KT, P, NR], F16, kind="ExternalInput")
    bal = nc.dram_tensor("bal", [RT, P, OUT], F16, kind="ExternalInput")
    mw = nc.dram_tensor("mw", [KT, P, OUT], F16, kind="ExternalInput")
    y = nc.dram_tensor("y", [BC, OUT], F16, kind="ExternalOutput")

    with tile.TileContext(nc) as tc:
        import contextlib

        loop_ctx = (
            tc.For_i(
                0,
                loop_n,
                1,
                staggered_reset=staggered,
                hint_engines=(
                    mybir.EngineType.PE,
                    mybir.EngineType.SP,
                    mybir.EngineType.Activation,
                    mybir.EngineType.DVE,
                    mybir.EngineType.Pool,
                ),
            )
            if loop_n is not None
            else contextlib.nullcontext()
        )
        with loop_ctx:
            with (
                tc.tile_pool(name="persist", bufs=1) as pp,
                tc.tile_pool(name="small", bufs=1) as sp,
                tc.tile_pool(name="mst", bufs=10) as mp,
                tc.tile_pool(name="bst", bufs=4) as bp,
                tc.tile_pool(name="ostg", bufs=3) as op_,
            ):
                ra_sb = sp.tile([P, RT], F32, name="ra_sb")
                wb_sb = sp.tile([P, BC], F16, name="wb_sb")
                xt_sb = pp.tile([P, KT, BC], F16, name="xt_sb")
                nc.sync.dma_start(out=xt_sb[:, 0:2, :], in_=xt.ap()[:, 0:2, :])
                nc.sync.dma_start(out=xt_sb[:, 2:8, :], in_=xt.ap()[:, 2:8, :])
                h_sb = pp.tile([P, RT, BC], F16, name="h_sb")
                # Phase H: H^T (all adapters), then mask -> h_sb (fp16).
                # k-outer with all 8 rank tiles accumulating in parallel PSUM
                # banks, so PE starts as soon as the first aal k-slice lands.
                psp = tc.alloc_tile_pool(name="psum", bufs=8, space="PSUM")
                with (
                    tc.tile_pool(name="ast", bufs=16) as ap_,
                    tc.tile_pool(name="maskp", bufs=8) as mkp,
                ):
                    msks = []
                    hpss = [
                        psp.tile([P, 512], F32, name=f"hps{rt}", tag="ps")[:, :BC]
                        for rt in range(RT)
                    ]
                    for k in range(KT):
                        if k == 6:
                            # mask inputs + masks: needed only at phase-H end
                            nc.gpsimd.dma_start(out=ra_sb[:], in_=ra.ap())
                            nc.gpsimd.dma_start(out=wb_sb[:], in_=wd.ap())
                            for rt in range(RT):
                                msk = mkp.tile(
                                    [P, BC], F16, name=f"msk{rt}", tag="msk"
                                )
                                nc.vector.tensor_scalar(
                                    out=msk[:],
                                    in0=wb_sb[:],
                                    scalar1=ra_sb[:, rt : rt + 1],
                                    scalar2=None,
                                    op0=mybir.AluOpType.is_equal,
                                )
                                msks.append(msk)
                        if k in (2, 10, 18):
                            kc = 8 * (k // 8 + 1)
                            nc.sync.dma_start(
                                out=xt_sb[:, kc : kc + 8, :],
                                in_=xt.ap()[:, kc : kc + 8, :],
                            )
                        at = ap_.tile([P, NR], F16, name="at", tag="at")
                        if k == 0:
                            # first tile split across both queues: halves the
                            # first matmul's data-arrival latency
                            nc.scalar.dma_start(
                                out=at[:, : NR // 2], in_=aal.ap()[0, :, : NR // 2]
                            )
                            nc.sync.dma_start(
                                out=at[:, NR // 2 :], in_=aal.ap()[0, :, NR // 2 :]
                            )
                        else:
                            eng = nc.scalar if k % 2 == 0 else nc.sync
                            eng.dma_start(out=at[:], in_=aal.ap()[k])
                        for rt in range(RT):
                            nc.tensor.matmul(
                                hpss[rt][:],
                                lhsT=at[:, rt * P : (rt + 1) * P],
                                rhs=xt_sb[:, k, :],
                                start=(k == 0),
                                stop=(k == KT - 1),
                            )
                    for rt in range(RT):
                        nc.vector.tensor_tensor(
                            out=h_sb[:, rt, :],
                            in0=hpss[rt][:],
                            in1=msks[rt][:],
                            op=mybir.AluOpType.mult,
                        )

                # Phase Y: y = x @ M + h_exp @ (2 * B_all), OUT in two halves.
                if True:
                    for h in range(NH):
                        ps = [
                            psp.tile([P, 512], F32, name=f"yps{h}_{j}", tag="ps")
                            for j in range(MB * NS)
                        ]
                        for k in range(KT):
                            mt = mp.tile([P, HW], F16, name="mt", tag="mt")
                            eng = nc.sync if k % 2 == 0 else nc.scalar
                            eng.dma_start(
                                out=mt[:], in_=mw.ap()[k, :, h * HW : (h + 1) * HW]
                            )
                            for mb in range(MB):
                                for ns in range(NS):
                                    nc.tensor.matmul(
                                        ps[mb * NS + ns][:],
                                        lhsT=xt_sb[:, k, mb * P : (mb + 1) * P],
                                        rhs=mt[:, ns * 512 : (ns + 1) * 512],
                                        start=(k == 0),
                                        stop=(k == KT - 1),
                                    )
                            # interleave one lora-B rank tile after every other
                            # M k-tile in the back half of the k-loop; spreads
                            # bt DMAs so the half-end has no load burst.
                            if k >= 8 and (k - 8) % 3 == 0:
                                rt = (k - 8) // 3
                                bt = bp.tile([P, HW], F16, name="bt", tag="bt")
                                eng = nc.sync if rt % 2 == 0 else nc.scalar
                                eng.dma_start(
                                    out=bt[:],
                                    in_=bal.ap()[rt, :, h * HW : (h + 1) * HW],
                                )
                                for mb in range(MB):
                                    for ns in range(NS):
                                        nc.tensor.matmul(
                                            ps[mb * NS + ns][:],
                                            lhsT=h_sb[:, rt, mb * P : (mb + 1) * P],
                                            rhs=bt[:, ns * 512 : (ns + 1) * 512],
                                            start=False,
                                            stop=False,
                                        )
                        for mb in range(MB):
                            ot = op_.tile([P, HW], F16, name="ot", tag="ot")
                            for ns in range(NS):
                                # split drains across DVE and ACT so the two
                                # banks' copies run in parallel at phase end
                                if ns % 2 == 0:
                                    nc.vector.tensor_copy(
                                        out=ot[:, ns * 512 : (ns + 1) * 512],
                                        in_=ps[mb * NS + ns][:],
                                    )
                                else:
                                    nc.scalar.copy(
                                        out=ot[:, ns * 512 : (ns + 1) * 512],
                                        in_=ps[mb * NS + ns][:],
                                    )
                                # ship each drained half as soon as it's ready
                                if ns == 1:
                                    nc.sync.dma_start(
                                        out=y.ap()[
                                            mb * P : (mb + 1) * P,
                                            h * HW : h * HW + 1024,
                                        ],
                                        in_=ot[:, :1024],
                                    )
                            nc.scalar.dma_start(
                                out=y.ap()[
                                    mb * P : (mb + 1) * P,
                                    h * HW + 1024 : (h + 1) * HW,
                                ],
                                in_=ot[:, 1024:],
                            )
                psp.release()

    nc.compile()
    return nc


def prep_inputs(x, wids, lora_A, lora_B, M):
    """Host-side sharding/layout prep. Returns per-core input maps."""
    x = np.asarray(x).reshape(B, IN).astype(np.float16, copy=False)
    wids = np.asarray(wids).reshape(B)
    lora_A = np.asarray(lora_A).astype(np.float16, copy=False)
    lora_B = np.asarray(lora_B).astype(np.float16, copy=False)
    M = np.asarray(M).astype(np.float16, copy=False)

    # [IN, NA*R]: column a*R+r is lora_A[a, :, r]
    aal_np = np.ascontiguousarray(
        lora_A.transpose(1, 0, 2).reshape(IN, NR).reshape(KT, P, NR)
    )
    # [NA*R, OUT] with the *2 output scale folded in (exact in fp16)
    bal_np = np.ascontiguousarray(
        (lora_B * np.float16(2.0)).reshape(NR, OUT).reshape(RT, P, OUT)
    )
    mw_np = np.ascontiguousarray(M.reshape(KT, P, OUT))
    ra_np = (
        (np.arange(RT)[None, :] * P + np.arange(P)[:, None]) // R
    ).astype(np.float32)

    in_maps = []
    for c in range(N_CORES):
        xs = x[c * BC : (c + 1) * BC]                      # [BC, IN]
        xt_np = np.ascontiguousarray(
            xs.T.reshape(KT, P, BC).transpose(1, 0, 2)
        )                                                  # [P, KT, BC]
        wd_np = np.ascontiguousarray(
            np.broadcast_to(
                wids[c * BC : (c + 1) * BC].astype(np.float16)[None, :], (P, BC)
            )
        )
        in_maps.append(
            {
                "xt": xt_np,
                "wd": wd_np,
                "ra": ra_np,
                "aal": aal_np,
                "bal": bal_np,
                "mw": mw_np,
            }
        )
    return in_maps


def kernel(x, wids, lora_A, lora_B, M):
    in_maps = prep_inputs(x, wids, lora_A, lora_B, M)
    nc = build_nc()
    res = run_bass_kernel_spmd(nc, in_maps, core_ids=list(range(N_CORES)))
    y = np.concatenate([res.results[c]["y"] for c in range(N_CORES)], axis=0)
    return y.reshape(B, 1, OUT)



# revision 2
# speedup vs baseline: 1.0137x; 1.0137x over previous
"""LoRA gather-BMM + dense GEMM kernel for Trainium2 (8 NeuronCores), v2.

Computation (per the module semantics):
    A = lora_A[wids]; Bw = lora_B[wids]
    y = (x @ A) @ Bw * 2 + x @ M          # x: [B, 1, IN]

Distribution: the host stable-sorts the batch by adapter id. Cores form a
4x2 grid: batch group g (512 sorted samples) x output-column half h
(2048 cols). Sorting makes each 128-sample subchunk span at most 8
distinct adapters (16 at sub=256 fallback), so the per-sample adapter
gather becomes a dense matmul against a tiny per-subchunk "local" bank
of 8 adapters (128 rank rows = one partition tile) followed by an
is_equal mask -- an 8x reduction in redundant LoRA FLOPs vs computing
all 64 adapters, and M traffic per core is halved by column sharding.

Per-core program (fp16 PE, fp32 PSUM):
  pass cs=0 (cols 0:512):   y-matmuls on 4 PSUM banks; interleaved in the
     same k-loop, rank activations H[t] = A_loc[t]^T @ x^T for the 4
     subchunk banks on the other 4 PSUM banks (same x k-tiles). At the
     end: mask H by (local_wid == slot) into fp16 h, then LoRA-B matmuls
     close out the pass-0 banks.
  passes cs=1..3: y-matmuls + mid-loop LoRA-B matmuls (h already ready).
No collectives; per-core outputs are stitched + unsorted on the host.

Benchmark loop mode: For_i inserts an all-engine semaphore-reset barrier
at the back edge (~8us: serial tail drains + cold DMA restart), so the
body is unrolled UNROLL x inside the loop -- internal body boundaries
overlap through data deps (x is held in three k-block tiles so a block's
rewrite only waits on that block's readers).
"""

import numpy as np

import concourse.bacc as bacc
import concourse.mybir as mybir
import concourse.tile as tile
from concourse.bass_utils import run_bass_kernel_spmd

B, IN, R, OUT, NA = 2048, 4096, 16, 4096, 64
N_CORES = 8
P = 128
KT = IN // P            # 32 contraction tiles over IN
PB = 4                  # batch shards
QC = 2                  # output-column shards
G = B // PB             # 512 samples per core
OUTC = OUT // QC        # 2048 cols per core
NT = G // P             # 4 rank tiles / sample tiles per core
CS = OUTC // 512        # 4 column-slice passes
MB = NT                 # 4 sample tiles of 128

F16 = mybir.dt.float16
F32 = mybir.dt.float32


def build_nc(loop_n=None, staggered=True, sub=128):
    """sub: sorted-subchunk size (128 default; 256 fallback when some
    128-window spans >8 distinct adapters)."""
    spt = sub // P          # partition tiles per subchunk (1 or 2)
    if loop_n is None:
        unroll = 1
    else:
        unroll = next((u for u in (40, 20, 8, 4, 2, 1) if loop_n % u == 0))

    nc = bacc.Bacc(
        "TRN2",
        target_bir_lowering=False,
        debug=False,
        enable_asserts=False,
        num_devices=N_CORES,
    )

    xt = nc.dram_tensor("xt", [P, KT, G], F16, kind="ExternalInput")
    wd = nc.dram_tensor("wd", [P, G], F16, kind="ExternalInput")
    ra = nc.dram_tensor("ra", [P, NT], F32, kind="ExternalInput")
    aal = nc.dram_tensor("aal", [P, KT, G], F16, kind="ExternalInput")
    bal = nc.dram_tensor("bal", [NT, P, OUTC], F16, kind="ExternalInput")
    mw = nc.dram_tensor("mw", [P, KT, OUTC], F16, kind="ExternalInput")
    y = nc.dram_tensor("y", [G, OUTC], F16, kind="ExternalOutput")

    with tile.TileContext(nc) as tc:
        import contextlib

        loop_ctx = (
            tc.For_i(
                0,
                loop_n // unroll,
                1,
                staggered_reset=staggered,
                hint_engines=(
                    mybir.EngineType.PE,
                    mybir.EngineType.SP,
                    mybir.EngineType.Activation,
                    mybir.EngineType.DVE,
                    mybir.EngineType.Pool,
                ),
            )
            if loop_n is not None
            else contextlib.nullcontext()
        )
        with loop_ctx:
            with (
                tc.tile_pool(name="persist", bufs=1) as pp,
                tc.tile_pool(name="small", bufs=2) as sp,
                tc.tile_pool(name="mst", bufs=12) as mp,
                tc.tile_pool(name="ast", bufs=2) as ap_,
                tc.tile_pool(name="bst", bufs=8) as bp,
                tc.tile_pool(name="maskp", bufs=8) as mkp,
                tc.tile_pool(name="ostg", bufs=8) as op_,
            ):
                # x in three k-block tiles: a block's rewrite next iteration
                # only waits for THAT block's readers (which finish early),
                # so the input stream overlaps the previous body's tail
                XBS = [(0, 8), (8, 20), (20, 32)]
                xbs = [
                    pp.tile([P, b - a, G], F16, name=f"xb{i}")
                    for i, (a, b) in enumerate(XBS)
                ]
                h_sb = pp.tile([P, NT, sub], F16, name="h_sb")

                def xk(k):
                    for i, (a, b) in enumerate(XBS):
                        if a <= k < b:
                            return xbs[i], k - a
                    raise AssertionError

                psp = tc.alloc_tile_pool(name="psum", bufs=8, space="PSUM")

                def body():
                    ra_sb = sp.tile([P, NT], F32, name="ra_sb", tag="ra")
                    wd_sb = sp.tile([P, G], F16, name="wd_sb", tag="wdt")

                    # boundary-critical DMAs all on gpsimd: its queue is idle
                    # during the previous body's tail, so these prefetch
                    # while sync/scalar are still shipping the prior outputs
                    nc.gpsimd.dma_start(
                        out=xbs[0][:, 0:2, :], in_=xt.ap()[:, 0:2, :]
                    )
                    at0 = ap_.tile([P, 1, G], F16, name="at0", tag="at0")
                    nc.gpsimd.dma_start(out=at0[:], in_=aal.ap()[:, 0:1, :])
                    mt0 = mp.tile([P, 2, 512], F16, name="mt", tag="mt")
                    nc.gpsimd.dma_start(out=mt0[:], in_=mw.ap()[:, 0:2, 0:512])
                    mt1 = mp.tile([P, 2, 512], F16, name="mt", tag="mt")
                    nc.gpsimd.dma_start(out=mt1[:], in_=mw.ap()[:, 2:4, 0:512])
                    at1 = ap_.tile([P, 3, G], F16, name="at1", tag="at1")
                    nc.gpsimd.dma_start(out=at1[:], in_=aal.ap()[:, 1:4, :])
                    at4a = ap_.tile([P, 4, G], F16, name="at4a", tag="at4a")
                    nc.gpsimd.dma_start(out=at4a[:], in_=aal.ap()[:, 4:8, :])
                    nc.gpsimd.dma_start(out=xbs[0][:, 2:8, :], in_=xt.ap()[:, 2:8, :])
                    nc.gpsimd.dma_start(out=xbs[1][:], in_=xt.ap()[:, 8:20, :])
                    nc.gpsimd.dma_start(out=xbs[2][:], in_=xt.ap()[:, 20:32, :])
                    nc.gpsimd.dma_start(out=wd_sb[:], in_=wd.ap())
                    nc.gpsimd.dma_start(out=ra_sb[:], in_=ra.ap())

                    # masks depend only on wd/ra: compute up front on DVE
                    msks = []
                    for t in range(NT):
                        sc = t // spt
                        msk = mkp.tile([P, sub], F16, name=f"msk{t}", tag="msk")
                        nc.vector.tensor_scalar(
                            out=msk[:],
                            in0=wd_sb[:, sc * sub : (sc + 1) * sub],
                            scalar1=ra_sb[:, t : t + 1],
                            scalar2=None,
                            op0=mybir.AluOpType.is_equal,
                        )
                        msks.append(msk)

                    hpss = None
                    at_cur, at_base = None, 0
                    for cs in range(CS):
                        ps = [
                            psp.tile([P, 512], F32, name=f"yps{cs}_{mb}", tag="ps")
                            for mb in range(MB)
                        ]
                        if cs == 0:
                            # created AFTER the pass-0 y banks: the next
                            # body's H matmuls then land on banks that
                            # drained two passes ago
                            hpss = [
                                psp.tile([P, 512], F32, name=f"hps{t}", tag="ps")[
                                    :, :sub
                                ]
                                for t in range(NT)
                            ]
                        bts = [None] * NT
                        for k in range(KT):
                            if k % 2 == 0:
                                # M stream: 2 k-tiles per DMA, alt queues
                                # (cs=0 k<4 pairs were loaded at the top)
                                if cs == 0 and k == 0:
                                    mt = mt0
                                elif cs == 0 and k == 2:
                                    mt = mt1
                                else:
                                    mt = mp.tile(
                                        [P, 2, 512], F16, name="mt", tag="mt"
                                    )
                                    eng = (
                                        nc.sync
                                        if (k // 2) % 2 == 0
                                        else nc.scalar
                                    )
                                    eng.dma_start(
                                        out=mt[:],
                                        in_=mw.ap()[
                                            :, k : k + 2, cs * 512 : (cs + 1) * 512
                                        ],
                                    )
                            if cs == 0:
                                # A-bank 4-k blocks (head blocks at top)
                                if k == 0:
                                    at_cur, at_base = at0, 0
                                elif k == 1:
                                    at_cur, at_base = at1, 1
                                elif k == 4:
                                    at_cur, at_base = at4a, 4
                                elif k % 4 == 0:
                                    at_cur = ap_.tile(
                                        [P, 4, G], F16, name="at4", tag="at4"
                                    )
                                    at_base = k
                                    eng = (
                                        nc.scalar
                                        if (k // 4) % 2 == 0
                                        else nc.sync
                                    )
                                    eng.dma_start(
                                        out=at_cur[:], in_=aal.ap()[:, k : k + 4, :]
                                    )
                                # LoRA-B tiles arrive late in pass 0
                                if k in (22, 24, 26, 28):
                                    t = (k - 22) // 2
                                    bts[t] = bp.tile(
                                        [P, 512], F16, name=f"bt{t}", tag="bt"
                                    )
                                    nc.gpsimd.dma_start(
                                        out=bts[t][:],
                                        in_=bal.ap()[t, :, cs * 512 : (cs + 1) * 512],
                                    )
                            elif k in (6, 10, 14, 18):
                                t = (k - 6) // 4
                                bts[t] = bp.tile(
                                    [P, 512], F16, name=f"bt{t}", tag="bt"
                                )
                                nc.gpsimd.dma_start(
                                    out=bts[t][:],
                                    in_=bal.ap()[t, :, cs * 512 : (cs + 1) * 512],
                                )
                            xb, kk = xk(k)
                            # k=0: y first (its banks freed two passes ago,
                            # so the next body's PE restarts instantly);
                            # later k: H first so the pass-0 tail masks
                            # overlap the final y k-tiles
                            order = ("y", "h") if k < 2 else ("h", "y")
                            for what in order:
                                if what == "h" and cs == 0:
                                    for t in range(NT):
                                        sc = t // spt
                                        nc.tensor.matmul(
                                            hpss[t][:],
                                            lhsT=at_cur[
                                                :, k - at_base, t * P : (t + 1) * P
                                            ],
                                            rhs=xb[:, kk, sc * sub : (sc + 1) * sub],
                                            start=(k == 0),
                                            stop=(k == KT - 1),
                                        )
                                elif what == "y":
                                    for mb in range(MB):
                                        nc.tensor.matmul(
                                            ps[mb][:],
                                            lhsT=xb[:, kk, mb * P : (mb + 1) * P],
                                            rhs=mt[:, k % 2, :],
                                            start=(k == 0),
                                            stop=(k == KT - 1 and cs != 0),
                                        )
                            if cs != 0 and k in (10, 14, 18, 22):
                                # h ready since pass 0: LoRA-B accumulation
                                t = (k - 10) // 4
                                sc = t // spt
                                for j in range(spt):
                                    nc.tensor.matmul(
                                        ps[sc * spt + j][:],
                                        lhsT=h_sb[:, t, j * P : (j + 1) * P],
                                        rhs=bts[t][:],
                                        start=False,
                                        stop=False,
                                    )
                        if cs == 0:
                            # mask the rank activations into fp16 h
                            for t in range(NT):
                                nc.vector.tensor_tensor(
                                    out=h_sb[:, t, :],
                                    in0=hpss[t][:],
                                    in1=msks[t][:],
                                    op=mybir.AluOpType.mult,
                                )
                            # LoRA-B matmuls close out the pass-0 banks
                            for t in range(NT):
                                sc = t // spt
                                for j in range(spt):
                                    nc.tensor.matmul(
                                        ps[sc * spt + j][:],
                                        lhsT=h_sb[:, t, j * P : (j + 1) * P],
                                        rhs=bts[t][:],
                                        start=False,
                                        stop=(t % spt == spt - 1),
                                    )
                        # drain + ship; final pass splits copies across
                        # DVE/ACT (the loop barrier waits on this tail)
                        for mb in range(MB):
                            ot = op_.tile([P, 512], F16, name="ot", tag="ot")
                            if cs == CS - 1 and mb % 2 == 1:
                                nc.scalar.copy(out=ot[:], in_=ps[mb][:])
                            else:
                                nc.vector.tensor_copy(out=ot[:], in_=ps[mb][:])
                            eng = nc.sync if mb % 2 == 0 else nc.scalar
                            eng.dma_start(
                                out=y.ap()[
                                    mb * P : (mb + 1) * P, cs * 512 : (cs + 1) * 512
                                ],
                                in_=ot[:],
                            )

                for _ in range(unroll):
                    body()
                psp.release()

    nc.compile()
    return nc


def _plan(wids):
    """Sort plan. Returns (order, sub) or (order, None) if even sub=256
    overflows (practically impossible for uniform wids)."""
    wids = np.asarray(wids).reshape(B)
    order = np.argsort(wids, kind="stable")
    ws = wids[order]
    for sub in (128, 256):
        ok = True
        for j in range(B // sub):
            if len(np.unique(ws[j * sub : (j + 1) * sub])) > sub // R:
                ok = False
                break
        if ok:
            return order, sub
    return order, None


def prep_inputs(x, wids, lora_A, lora_B, M, sub=None):
    """Host-side sharding/layout prep. Returns per-core input maps."""
    x = np.asarray(x).reshape(B, IN).astype(np.float16, copy=False)
    wids = np.asarray(wids).reshape(B)
    lora_A = np.asarray(lora_A).astype(np.float16, copy=False)
    lora_B = np.asarray(lora_B).astype(np.float16, copy=False)
    M = np.asarray(M).astype(np.float16, copy=False)

    order, auto_sub = _plan(wids)
    if sub is None:
        sub = auto_sub
    assert sub is not None, "adapter span overflow; use numpy fallback"
    nal = sub // R
    spt = sub // P
    ws = wids[order]

    ra_np = np.ascontiguousarray(
        (
            ((np.arange(NT)[None, :] % spt) * P + np.arange(P)[:, None]) // R
        ).astype(np.float32)
    )
    mw_halves = [
        np.ascontiguousarray(
            M[:, h * OUTC : (h + 1) * OUTC].reshape(KT, P, OUTC).transpose(1, 0, 2)
        )
        for h in range(QC)
    ]

    in_maps = []
    for g in range(PB):
        idx = order[g * G : (g + 1) * G]
        xt_np = np.ascontiguousarray(
            x[idx].T.reshape(KT, P, G).transpose(1, 0, 2)
        )
        a_cols = np.empty((IN, G), np.float16)
        b_rows = np.empty((G, OUT), np.float16)
        lw = np.empty(G, np.float16)
        for sc in range(G // sub):
            wc = ws[g * G + sc * sub : g * G + (sc + 1) * sub]
            uq = np.unique(wc)
            lw[sc * sub : (sc + 1) * sub] = np.searchsorted(uq, wc)
            uqp = np.concatenate([uq, np.zeros(nal - len(uq), uq.dtype)])
            a_cols[:, sc * sub : (sc + 1) * sub] = (
                lora_A[uqp].transpose(1, 0, 2).reshape(IN, sub)
            )
            b_rows[sc * sub : (sc + 1) * sub] = (
                lora_B[uqp].reshape(sub, OUT) * np.float16(2.0)
            )
        aal_np = np.ascontiguousarray(
            a_cols.reshape(KT, P, G).transpose(1, 0, 2)
        )
        bal_np = b_rows.reshape(NT, P, OUT)
        wd_np = np.ascontiguousarray(np.broadcast_to(lw[None, :], (P, G)))
        for h in range(QC):
            in_maps.append(
                {
                    "xt": xt_np,
                    "wd": wd_np,
                    "ra": ra_np,
                    "aal": aal_np,
                    "bal": np.ascontiguousarray(
                        bal_np[:, :, h * OUTC : (h + 1) * OUTC]
                    ),
                    "mw": mw_halves[h],
                }
            )
    return in_maps


def _kernel_numpy(x, wids, lora_A, lora_B, M):
    x2 = np.asarray(x, np.float32).reshape(B, IN)
    A = np.asarray(lora_A, np.float32)[wids]
    Bw = np.asarray(lora_B, np.float32)[wids]
    h = np.einsum("bi,bir->br", x2, A)
    y = np.einsum("br,bro->bo", h, Bw) * 2.0 + x2 @ np.asarray(M, np.float32)
    return y.astype(np.float16).reshape(B, 1, OUT)


def kernel(x, wids, lora_A, lora_B, M):
    order, sub = _plan(wids)
    if sub is None:
        return _kernel_numpy(x, np.asarray(wids).reshape(B), lora_A, lora_B, M)
    in_maps = prep_inputs(x, wids, lora_A, lora_B, M, sub=sub)
    nc = build_nc(sub=sub)
    res = run_bass_kernel_spmd(nc, in_maps, core_ids=list(range(N_CORES)))
    ys = np.empty((B, OUT), np.float16)
    for c in range(N_CORES):
        g, h = c // QC, c % QC
        ys[g * G : (g + 1) * G, h * OUTC : (h + 1) * OUTC] = res.results[c]["y"]
    y_full = np.empty_like(ys)
    y_full[order] = ys
    return y_full.reshape(B, 1, OUT)
